# revision 2
# baseline (speedup 1.0000x reference)
"""Causal self-attention (GQA + rope + rms-norm + int4 fake-quant weights) on 8 trn2 cores.

Sharding: core = (batch b, kv-group g); b = core // 4, g = core % 4.
Each core computes heads 4g..4g+3 of batch b through attention, AllGathers
y.T (bf16) across its 4-core batch group, and produces the output projection
slice out[b, :, 256g:256g+256] (w_proj row-split keeps per-row int4 quant
exact). Host does slicing / transposes / dtype casts / concat only.

v2 design:
- bf16 matmul feeds + DVE elementwise (2x DVE modes, half DMA bytes); int4
  fake-quant math stays f32, dequant emits bf16.
- XBAR DMA transposes (dma_start_transpose) build wT and vAug off-engine.
- rms factor rsqrt = exp(-0.5*ln(ssq/HD+eps)) so every ACT op (square, copy,
  ln, exp) lives in the one natural_log_exp_and_others table: no table loads.
- q_gain/8 baked into the host-built bd2 broadcast matrices; softmax needs no
  extra scaling anywhere.
- rope swap halves via a PE permutation matmul; rms factor premultiplied into
  raw before rotation (factor is constant within each head so it commutes).
- attention scores computed per j-PAIR into 2-bank PSUM tiles; one exp per
  pair; causal masks via bf16 dmask (even tile) / dmask2 (odd tile) mults.
- software pipeline per chunk c: outproj(c-1) | attn(c) head blocks
  interleaved with proj(c+1) stages | AllGather(c).
- output stored bf16, upcast on host.
"""

import sys

sys.path.insert(0, "/opt/trn_rl_repo")

import functools
import numpy as np
import ml_dtypes

import jax

jax.config.update("jax_compilation_cache_dir", "/tmp/jax_cache")
jax.config.update("jax_persistent_cache_min_entry_size_bytes", -1)
jax.config.update("jax_persistent_cache_min_compile_time_secs", 0)

import concourse.bass as bass
import concourse.mybir as mybir
import concourse.tile as tile
from concourse import bacc
from concourse.bass_utils import run_bass_kernel_spmd
from concourse.hw_specs import get_activation_tables

F32 = mybir.dt.float32
BF16 = mybir.dt.bfloat16
AF = mybir.ActivationFunctionType
ALU = mybir.AluOpType
BF = ml_dtypes.bfloat16

B, S, D = 2, 2048, 1024
H, KVH, HD = 16, 4, 64
G = 4
N_CORES = 8
P = 128
CH = 512
NCH = S // CH  # 4
KT = D // P  # 8
QROWS = H // G * HD  # 256
EPS = 1.1920929e-7
MAGIC = 12582912.0  # 1.5*2**23
ROPE_BASE = 10000.0
C17 = 0.14285714285714285


def build_nc(n_cores=N_CORES, group_size=G, phases=9, no_cc=False, repeat=1,
             dbg=False, **_ignored):
    nc = bacc.Bacc("TRN2", target_bir_lowering=False, debug=False, num_devices=n_cores)
    groups = [list(range(s, s + group_size)) for s in range(0, n_cores, group_size)]

    xT_in = nc.dram_tensor("xT", [D, S], BF16, kind="ExternalInput").ap()
    wq_in = nc.dram_tensor("wq", [QROWS, D], F32, kind="ExternalInput").ap()
    wkv_in = nc.dram_tensor("wkv", [2 * HD, D], F32, kind="ExternalInput").ap()
    wp_in = nc.dram_tensor("wp", [QROWS, D], F32, kind="ExternalInput").ap()
    cos2_in = nc.dram_tensor("cos2", [P, S], BF16, kind="ExternalInput").ap()
    sin2_in = nc.dram_tensor("sin2", [P, S], BF16, kind="ExternalInput").ap()
    bd_in = nc.dram_tensor("bd", [P, 2], BF16, kind="ExternalInput").ap()
    ones64_in = nc.dram_tensor("ones64", [HD, 1], BF16, kind="ExternalInput").ap()
    onesrow_in = nc.dram_tensor("onesrow", [1, HD], BF16, kind="ExternalInput").ap()
    bd2q_in = nc.dram_tensor("bd2q", [4, P], BF16, kind="ExternalInput").ap()
    swapm_in = nc.dram_tensor("swapm", [P, P], BF16, kind="ExternalInput").ap()
    dmask_in = nc.dram_tensor("dmask", [P, P], BF16, kind="ExternalInput").ap()
    dmask2_in = nc.dram_tensor("dmask2", [P, 2 * P], BF16, kind="ExternalInput").ap()
    out = nc.dram_tensor("out", [S, QROWS], BF16, kind="ExternalOutput").ap()
    dbg_out = (
        {
            "qTr0": nc.dram_tensor("dbg_qTr0", [P, S], BF16, kind="ExternalOutput").ap(),
            "kTr": nc.dram_tensor("dbg_kTr", [P, S], BF16, kind="ExternalOutput").ap(),
            "vAug": nc.dram_tensor("dbg_vAug", [P, (S // P) * (HD + 1)], BF16, kind="ExternalOutput").ap(),
            "cc": nc.dram_tensor("dbg_cc", [G * HD, S], BF16, kind="ExternalOutput").ap(),
            "rfac": nc.dram_tensor("dbg_rfac", [65, S], BF16, kind="ExternalOutput").ap(),
        }
        if dbg
        else None
    )

    with tile.TileContext(nc) as tc:
        with (
            tc.tile_pool(name="consts", bufs=1) as cp,
            tc.tile_pool(name="nat", bufs=2) as natp,
            tc.tile_pool(name="wT", bufs=1) as wtp,
            tc.tile_pool(name="xt", bufs=1) as xtp,
            tc.tile_pool(name="persist", bufs=1) as pp,
            tc.tile_pool(name="work", bufs=2) as wk,
            tc.tile_pool(name="dram", bufs=4, space="DRAM") as dp,
            tc.tile_pool(name="ps_mm", bufs=3, space="PSUM") as ps_mm,
            tc.tile_pool(name="ps_po", bufs=2, space="PSUM") as ps_po,
        ):
            # ---- constants ----
            cos2 = cp.tile([P, S], BF16, tag="cos2")
            nc.sync.dma_start(cos2[:], cos2_in[:])
            sin2 = cp.tile([P, S], BF16, tag="sin2")
            nc.sync.dma_start(sin2[:], sin2_in[:])
            bd = cp.tile([P, 2], BF16, tag="bd")
            nc.sync.dma_start(bd[:], bd_in[:])
            ones64 = cp.tile([HD, 1], BF16, tag="ones64")
            nc.sync.dma_start(ones64[:], ones64_in[:])
            # ones row replicated at partitions 0 (tail broadcast) and 64
            # (k-factor broadcast): matmul lhsT/rhs bases must match
            onesrow = cp.tile([65, HD], BF16, tag="onesrow")
            nc.sync.dma_start(onesrow[0:1, :], onesrow_in[:])
            nc.sync.dma_start(onesrow[64:65, :], onesrow_in[:])
            # bd2 gain rows at partition bases 0 and 32 to match rfac slices
            bd2qt = cp.tile([34, P], BF16, tag="bd2qt")
            nc.sync.dma_start(bd2qt[0:2, :], bd2q_in[0:2, :])
            nc.sync.dma_start(bd2qt[32:34, :], bd2q_in[2:4, :])
            swapm = cp.tile([P, P], BF16, tag="swapm")
            nc.sync.dma_start(swapm[:], swapm_in[:])
            dmask = cp.tile([P, P], BF16, tag="dmask")
            nc.sync.dma_start(dmask[:], dmask_in[:])
            dmask2 = cp.tile([P, 2 * P], BF16, tag="dmask2")
            nc.sync.dma_start(dmask2[:], dmask2_in[:])
            epsb = cp.tile([65, 1], F32, tag="epsb")
            nc.any.memset(epsb[:], EPS)

            # pin the one act table holding Copy/Square/Ln/Exp so the greedy
            # table-load pass never thrashes between per-func tables
            need = {AF.Copy, AF.Square, AF.Ln, AF.Exp}
            tabs = list(get_activation_tables(nc.m.arch).items())
            tid = next(i for i, (_, fs) in enumerate(tabs) if need <= fs)
            nc.scalar.add_instruction(mybir.InstLoadActFuncSet(
                name=f"I-{nc.next_id()}", ins=[], outs=[], act_func_set_id=tid))

            for rep in range(repeat):
                _iteration(
                    nc, tc, rep, phases, no_cc, groups, group_size,
                    xT_in, wq_in, wkv_in, wp_in, out,
                    cos2, sin2, bd, ones64, onesrow, bd2qt, swapm,
                    dmask, dmask2, epsb,
                    natp, wtp, xtp, pp, wk, dp, ps_mm, ps_po, dbg_out,
                )

    nc.compile()
    return nc


def _iteration(
    nc, tc, rep, phases, no_cc, groups, group_size,
    xT_in, wq_in, wkv_in, wp_in, out,
    cos2, sin2, bd, ones64, onesrow, bd2qt, swapm, dmask, dmask2, epsb,
    natp, wtp, xtp, pp, wk, dp, ps_mm, ps_po, dbg_out=None,
):
    # ---- persistent tiles ----
    wqT = wtp.tile([P, KT, QROWS], BF16, tag="wqT", name=f"r{rep}_wqT")
    wkvT = wtp.tile([P, KT, P], BF16, tag="wkvT", name=f"r{rep}_wkvT")
    wpT = wtp.tile([P, KT, QROWS], BF16, tag="wpT", name=f"r{rep}_wpT")
    xt = {}
    for c in range(NCH):
        for k in range(KT):
            xt[(k, c)] = xtp.tile(
                [P, CH], BF16, tag=f"xt{k}_{c}", name=f"r{rep}_xt{k}_{c}"
            )
    qTr = [
        pp.tile([P, S], BF16, tag=f"qTr{i}", name=f"r{rep}_qTr{i}") for i in range(2)
    ]
    kTr = pp.tile([P, S], BF16, tag="kTr", name=f"r{rep}_kTr")
    vAug = pp.tile([P, S // P, HD + 1], BF16, tag="vAug", name=f"r{rep}_vAug")
    nc.any.memset(vAug[:, :, HD : HD + 1], 1.0)

    # ---- x loads: chunk 0 first so proj(0) starts early; the rest are
    # emitted after the first quant blocks (same sync queue, ordered) ----
    def load_xt(c):
        for k in range(KT):
            nc.sync.dma_start(
                xt[(k, c)][:],
                xT_in[k * P : (k + 1) * P, c * CH : (c + 1) * CH],
            )

    load_xt(0)

    # ---- weight int4 fake-quant + XBAR transpose ----
    def quant_block(src, row0, dstT, dcol0):
        w_nat = natp.tile([P, D], F32, tag="w_nat", bufs=2)
        nc.gpsimd.dma_start(w_nat[:], src[row0 : row0 + P, :])
        m = wk.tile([P, 1], F32, tag="q_m", bufs=2)
        nc.vector.tensor_reduce(
            m[:], w_nat[:], axis=mybir.AxisListType.X, op=ALU.max,
            apply_absolute_value=True,
        )
        nc.vector.tensor_scalar(m[:], m[:], 1e-8, None, ALU.max)
        scale = wk.tile([P, 1], F32, tag="q_scale", bufs=2)
        nc.vector.tensor_scalar(scale[:], m[:], C17, None, ALU.mult)
        rsc = wk.tile([P, 1], F32, tag="q_rsc", bufs=2)
        with nc.allow_low_precision(reason="quant reciprocal"):
            nc.vector.reciprocal(rsc[:], scale[:])
        # one Newton step: rsc *= (2 - scale*rsc)
        nt = wk.tile([P, 1], F32, tag="q_nt", bufs=2)
        nc.vector.tensor_scalar_mul(nt[:], rsc[:], scale[:])
        nc.vector.tensor_scalar(nt[:], nt[:], -1.0, 2.0, ALU.mult, ALU.add)
        nc.vector.tensor_scalar_mul(rsc[:], rsc[:], nt[:])
        # round pass (ACT): t = w*rsc + MAGIC; dequant (DVE): (t - MAGIC)*scale
        tmag = wk.tile([P, D], F32, tag="q_tmag", bufs=2)
        nc.gpsimd.tensor_scalar(tmag[:], w_nat[:], rsc[:], MAGIC, ALU.mult,
                                ALU.add)
        wdq = wk.tile([P, D], BF16, tag="q_wdq", bufs=2)
        nc.vector.tensor_scalar(wdq[:], tmag[:], -MAGIC, scale[:], ALU.add, ALU.mult)
        nc.scalar.dma_start_transpose(dstT[:, :, dcol0 : dcol0 + P], wdq[:])

    quant_block(wq_in, 0, wqT, 0)
    quant_block(wq_in, P, wqT, P)
    quant_block(wkv_in, 0, wkvT, 0)
    for c in range(1, NCH):
        load_xt(c)

    # ---- projections + rms + rope for one chunk ----
    def proj_stage1(c):
        """pq/pkv matmuls, squares, ssq, rsqrt, fb broadcasts.
        Returns tiles needed by stage2."""
        sl = slice(c * CH, (c + 1) * CH)
        pqp = ps_mm.tile([P, 2, CH], F32, tag="mm", name=f"r{rep}_pqp{c}")
        for half in range(2):
            for k in range(KT):
                nc.tensor.matmul(
                    pqp[:, half, :],
                    wqT[:, k, half * P : (half + 1) * P],
                    xt[(k, c)][:],
                    start=(k == 0), stop=(k == KT - 1),
                )
        pkv = ps_mm.tile([P, 2, CH], F32, tag="mm", name=f"r{rep}_pkv{c}")
        for k in range(KT):
            nc.tensor.matmul(
                pkv[:, 0, :], wkvT[:, k, :], xt[(k, c)][:],
                start=(k == 0), stop=(k == KT - 1),
            )
        # raw q out of PSUM once (ACT); square on DVE from SBUF (TT may read
        # at most one PSUM operand, so raw also feeds the rms premultiply)
        raw = wk.tile([P, 2, CH], BF16, tag="raw", bufs=2)
        nc.scalar.activation(
            raw[:].rearrange("p a b -> p (a b)"),
            pqp[:].rearrange("p a b -> p (a b)"),
            AF.Copy,
        )
        q2p = wk.tile([P, 2, CH], BF16, tag="q2p", bufs=2)
        nc.vector.tensor_mul(
            q2p[:].rearrange("p a b -> p (a b)"),
            raw[:].rearrange("p a b -> p (a b)"),
            raw[:].rearrange("p a b -> p (a b)"),
        )
        q2k = wk.tile([HD, CH], BF16, tag="q2k", bufs=2)
        nc.scalar.activation(q2k[:], pkv[:HD, 0, :], AF.Square)
        # sum of squares into pkv's unused half; matmul outs must base at
        # partition 0/32/64, so rows live at {0,1}, {32,33}, {64}
        ssq = pkv[0:65, 1, :]
        nc.tensor.matmul(ssq[0:2], bd[:], q2p[:, 0, :], start=True, stop=True,
                         skip_group_check=True)
        nc.tensor.matmul(ssq[32:34], bd[:], q2p[:, 1, :], start=True, stop=True,
                         skip_group_check=True)
        nc.tensor.matmul(ssq[64:65], ones64[:], q2k[:], start=True, stop=True,
                         skip_group_check=True)
        # rsqrt chain entirely in the exp/ln table: r = exp(-0.5*ln(x/HD+eps));
        # unused partitions compute ln of stale psum -- never read downstream
        lnt = wk.tile([65, CH], F32, tag="lnt", bufs=2)
        nc.scalar.activation(lnt[:], ssq, AF.Ln, bias=epsb[:], scale=1.0 / HD)
        rfac = wk.tile([65, CH], BF16, tag="rfac", bufs=2)
        nc.scalar.activation(rfac[:], lnt[:], AF.Exp, scale=-0.5)
        # broadcast factors to full partition height (gains/8 baked into bd2q)
        fbp = ps_mm.tile([P, 2, CH], F32, tag="mm", name=f"r{rep}_fbp{c}")
        nc.tensor.matmul(fbp[:, 0, :], bd2qt[0:2, :], rfac[0:2, :], start=True,
                         stop=True, skip_group_check=True)
        nc.tensor.matmul(fbp[:, 1, :], bd2qt[32:34, :], rfac[32:34, :], start=True,
                         stop=True, skip_group_check=True)
        # k factor broadcast into pkv's free bank-1 partitions, then an ACT
        # cross-partition copy down to partitions 0-63 for the aligned TT mul
        nc.tensor.matmul(pkv[HD : 2 * HD, 1, :], onesrow[64:65, :],
                         rfac[64:65, :], start=True, stop=True,
                         skip_group_check=True)
        fbks = wk.tile([HD, CH], BF16, tag="fbks", bufs=2)
        nc.scalar.activation(fbks[:], pkv[HD : 2 * HD, 1, :], AF.Copy)
        if dbg_out is not None:
            nc.sync.dma_start(dbg_out["rfac"][:, c * CH : (c + 1) * CH], rfac[:])
        # v half -> bf16 (ACT copy), XBAR-transposed into a contiguous staging
        # tile (the xbar mis-writes gapped out APs), then DVE-copied into the
        # ones-augmented vAug layout
        vb = wk.tile([HD, CH], BF16, tag="vb", bufs=2)
        nc.scalar.activation(vb[:], pkv[HD:, 0, :], AF.Copy)
        vT = wk.tile([P, CH // P, HD], BF16, tag="vT", bufs=2)
        nc.sync.dma_start_transpose(vT[:], vb[:])
        nc.vector.tensor_copy(
            vAug[:, c * (CH // P) : (c + 1) * (CH // P), 0:HD], vT[:]
        )
        return sl, raw, pkv, fbp, fbks

    def proj_stage2(c, st1):
        """rms-premultiply, rope rotation, qTr/kTr writes."""
        sl, raw, pkv, fbp, fbks = st1
        # rawf = raw * rsqrt * gain/8  (factor constant per head => commutes
        # with the rotation)
        rawf = wk.tile([P, 2, CH], BF16, tag="rawf", bufs=2)
        nc.vector.tensor_mul(
            rawf[:].rearrange("p a b -> p (a b)"),
            raw[:].rearrange("p a b -> p (a b)"),
            fbp[:].rearrange("p a b -> p (a b)"),
        )
        rawfk = wk.tile([HD, CH], BF16, tag="rawfk", bufs=2)
        nc.vector.tensor_mul(rawfk[:], pkv[:HD, 0, :], fbks[:])
        # swap halves on PE (all pkv/pqp/fbp readers are emitted above, so the
        # mm ring may recycle their slots here)
        swq = ps_mm.tile([P, 2, CH], F32, tag="mm", name=f"r{rep}_swq{c}")
        for half in range(2):
            nc.tensor.matmul(
                swq[:, half, :], swapm[:], rawf[:, half, :],
                start=True, stop=True, skip_group_check=True,
            )
        swk = ps_mm.tile([P, 2, CH], F32, tag="mm", name=f"r{rep}_swk{c}")
        nc.tensor.matmul(swk[:HD, 0, :], swapm[:HD, :HD], rawfk[:],
                         start=True, stop=True, skip_group_check=True)
        # rot = rawf*cos + swap(rawf)*sin
        cosb = cos2[:, None, sl].to_broadcast((P, 2, CH))
        sinb = sin2[:, None, sl].to_broadcast((P, 2, CH))
        t2 = wk.tile([P, 2, CH], BF16, tag="t2", bufs=2)
        nc.vector.tensor_mul(t2[:], rawf[:], cosb)
        qs = wk.tile([P, 2, CH], BF16, tag="qs", bufs=2)
        nc.vector.tensor_mul(qs[:], swq[:], sinb)
        for half in range(2):
            nc.vector.tensor_add(qTr[half][:, sl], qs[:, half, :], t2[:, half, :])
        t2k = wk.tile([HD, CH], BF16, tag="t2k", bufs=2)
        nc.vector.tensor_mul(t2k[:], rawfk[:], cos2[:HD, sl])
        qsk = wk.tile([HD, CH], BF16, tag="qsk", bufs=2)
        nc.vector.tensor_mul(qsk[:], swk[:HD, 0, :], sin2[:HD, sl])
        nc.vector.tensor_add(kTr[:HD, sl], qsk[:], t2k[:])
        # duplicate k rows at partitions 64-127 so odd heads' score matmuls
        # get a lhsT whose base partition matches their rhs (qTr upper half)
        nc.gpsimd.tensor_copy(kTr[HD:P, sl], kTr[:HD, sl])

    st1 = proj_stage1(0)
    proj_stage2(0, st1)
    quant_block(wp_in, 0, wpT, 0)
    quant_block(wp_in, P, wpT, P)

    # ---- attention / collective / outproj pipeline ----
    def attn_head(c, h, po):
        """scores, exp, masks, attn@v for one head of one chunk."""
        qv = qTr[h // 2][h % 2 * HD : (h % 2 + 1) * HD, :]
        npair = 2 * c + 2
        pend = None  # (et, f0e, f0o, first) awaiting attn@v
        for pr_i in range(npair):
            j0 = 2 * pr_i
            diag = j0 >= 4 * c
            r0 = j0 - 4 * c
            f0e = r0 * P if diag else 0
            f0o = (r0 + 1) * P if diag else 0
            psc = ps_mm.tile([P, 2, CH], F32, tag="mm",
                             name=f"r{rep}_psc{c}_{h}_{pr_i}")
            # odd tile also computed from f0e (not f0o): the extra 128 cols are
            # above-diagonal junk that exp keeps finite and dmask2 zeroes;
            # starting both at f0e lets one 3D exp cover the pair with no
            # stale-PSUM reads.
            kb = h % 2 * HD
            nc.tensor.matmul(
                psc[:, 0, f0e:], kTr[kb : kb + HD, j0 * P : (j0 + 1) * P],
                qv[:, c * CH + f0e : (c + 1) * CH],
                start=True, stop=True, skip_group_check=True,
            )
            nc.tensor.matmul(
                psc[:, 1, f0e:], kTr[kb : kb + HD, (j0 + 1) * P : (j0 + 2) * P],
                qv[:, c * CH + f0e : (c + 1) * CH],
                start=True, stop=True, skip_group_check=True,
            )
            if pend is not None:
                _avpair(c, po, *pend)
            et = wk.tile([P, 2, CH], BF16, tag="et", bufs=3)
            nc.scalar.activation(et[:, :, f0e:], psc[:, :, f0e:], AF.Exp)
            if diag:
                nc.vector.tensor_mul(
                    et[:, 0, f0e : f0e + P], et[:, 0, f0e : f0e + P], dmask[:]
                )
                nc.vector.tensor_mul(
                    et[:, 1, f0e : f0e + 2 * P], et[:, 1, f0e : f0e + 2 * P],
                    dmask2[:],
                )
            pend = (et, f0e, f0o, j0, pr_i == 0, pr_i == npair - 1)
        _avpair(c, po, *pend)

    def _avpair(c, po, et, f0e, f0o, j0, first, last):
        nc.tensor.matmul(
            po[: HD + 1, f0e:], vAug[:, j0, :], et[:, 0, f0e:],
            start=first, stop=False, skip_group_check=True,
        )
        nc.tensor.matmul(
            po[: HD + 1, f0o:], vAug[:, j0 + 1, :], et[:, 1, f0o:],
            start=False, stop=last, skip_group_check=True,
        )

    def tail(po, cc_buf, row, c, h):
        rs = wk.tile([1, CH], BF16, tag="rs", bufs=2)
        with nc.allow_low_precision(reason="softmax denominator"):
            nc.vector.reciprocal(rs[:], po[HD : HD + 1, :])
        pr = ps_mm.tile([P, 2, CH], F32, tag="mm", name=f"r{rep}_pr{c}_{h}")
        nc.tensor.matmul(pr[:HD, 0, :], onesrow[0:1, :], rs[:], start=True,
                         stop=True, skip_group_check=True)
        rb = wk.tile([HD, CH], BF16, tag="rb", bufs=2)
        nc.vector.tensor_copy(rb[:], pr[:HD, 0, :])
        yt = wk.tile([HD, CH], BF16, tag="yt", bufs=2)
        nc.vector.tensor_mul(yt[:], po[:HD, :], rb[:])
        nc.sync.dma_start(cc_buf[row : row + HD, :], yt[:])
        if dbg_out is not None:
            nc.sync.dma_start(
                dbg_out["cc"][h * HD : (h + 1) * HD, c * CH : (c + 1) * CH],
                yt[:],
            )

    def emit_cc(cin, cout, rows):
        if no_cc:
            for gg in range(group_size):
                nc.sync.dma_start(cout[gg * rows : (gg + 1) * rows, :], cin[:])
        else:
            # on the DVE queue: the pool queue carries the ya reloads that
            # WAIT on the previous collective, which would head-of-line block
            # the next collective's issue
            nc.gpsimd.collective_compute(
                "AllGather",
                ALU.bypass,
                replica_groups=groups,
                ins=[cin.opt()],
                outs=[cout.opt()],
            )

    def outproj(c, ya_pairs):
        n = len(ya_pairs)
        for pfi in range(2):
            pf = ps_po.tile([P, CH], F32, tag="po", name=f"r{rep}_pf{c}_{pfi}")
            for sh in range(2):
                shg = pfi * 2 + sh
                for idx, (yat, gk) in enumerate(ya_pairs):
                    nc.tensor.matmul(
                        pf[:, sh * QROWS : (sh + 1) * QROWS],
                        yat[:, shg * P : (shg + 1) * P],
                        wpT[:, gk, :],
                        start=(idx == 0), stop=(idx == n - 1),
                        skip_group_check=True,
                    )
            ot = wk.tile([P, CH], BF16, tag="ot", bufs=2)
            nc.vector.tensor_copy(ot[:], pf[:])
            base = c * CH + pfi * 2 * P
            nc.gpsimd.dma_start(
                out[base : base + 2 * P, :].rearrange("(a p) d -> p a d", a=2),
                ot[:].rearrange("p (a d) -> p a d", a=2),
            )

    # emission template per chunk:
    #   [attn h0 | stage1(c+1)] [tail h0 | stage2(c+1) | attn h1]
    #   [tail h1 | outproj(c-1) | attn h2] [tail h2 | attn h3]
    #   [tail h3 | CC | ya loads]
    # one AllGather per chunk (split in two for the last chunk so half is in
    # flight while heads 2/3 still compute); tails precede attn blocks so the
    # pf po-ring waits never head-of-line block PE behind unmet DVE work
    pending = None  # (chunk, ya_pairs) with deferred output projection
    for c in range(NCH):
        last = c == NCH - 1
        if last:
            cc_a_in = dp.tile([2 * HD, CH], BF16, tag="cca_in",
                              name=f"r{rep}_ccain{c}")
            cc_a_out = dp.tile([G * 2 * HD, CH], BF16, tag="cca_out",
                               name=f"r{rep}_ccaout{c}")
            cc_b_in = dp.tile([2 * HD, CH], BF16, tag="ccb_in",
                              name=f"r{rep}_ccbin{c}")
            cc_b_out = dp.tile([G * 2 * HD, CH], BF16, tag="ccb_out",
                               name=f"r{rep}_ccbout{c}")
            tgt = lambda h: ((cc_a_in, h * HD) if h < 2
                             else (cc_b_in, (h - 2) * HD))
        else:
            cc_in = dp.tile([G * HD, CH], BF16, tag="cc_in",
                            name=f"r{rep}_ccin{c}")
            cc_out = dp.tile([G * G * HD, CH], BF16, tag="cc_out",
                             name=f"r{rep}_ccout{c}")
            tgt = lambda h: (cc_in, h * HD)

        st1 = None
        pos = [None] * 4
        for h in range(4):
            if h >= 1:
                buf, row = tgt(h - 1)
                tail(pos[h - 1], buf, row, c, h - 1)
            if h == 2:
                if last:
                    emit_cc(cc_a_in, cc_a_out, 2 * HD)
                if pending is not None:
                    outproj(*pending)
                    pending = None
            if h == 1 and st1 is not None:
                proj_stage2(c + 1, st1)
            po = ps_po.tile([P, CH], F32, tag="po", name=f"r{rep}_po{c}_{h}")
            pos[h] = po
            attn_head(c, h, po)
            if h == 0 and c + 1 < NCH:
                st1 = proj_stage1(c + 1)
        buf, row = tgt(3)
        tail(pos[3], buf, row, c, 3)
        ya = [
            wk.tile([P, CH], BF16, tag="ya", bufs=8, name=f"r{rep}_ya{c}_{k}")
            for k in range(KT)
        ]
        if last:
            emit_cc(cc_b_in, cc_b_out, 2 * HD)
            for i in range(G):
                nc.gpsimd.dma_start(
                    ya[2 * i][:], cc_a_out[i * P : (i + 1) * P, :]
                )
            for i in range(G):
                nc.gpsimd.dma_start(
                    ya[2 * i + 1][:], cc_b_out[i * P : (i + 1) * P, :]
                )
            pairs = [(ya[2 * i], 2 * i) for i in range(G)] + [
                (ya[2 * i + 1], 2 * i + 1) for i in range(G)
            ]
        else:
            emit_cc(cc_in, cc_out, G * HD)
            for k in range(KT):
                nc.gpsimd.dma_start(ya[k][:], cc_out[k * P : (k + 1) * P, :])
            pairs = [(ya[k], k) for k in range(KT)]
        pending = (c, pairs)
    outproj(*pending)
    if dbg_out is not None:
        nc.sync.dma_start(dbg_out["qTr0"][:], qTr[0][:])
        nc.sync.dma_start(dbg_out["kTr"][:], kTr[:])
        nc.sync.dma_start(
            dbg_out["vAug"][:], vAug[:].rearrange("p a b -> p (a b)")
        )


@functools.lru_cache(maxsize=None)
def get_nc():
    return build_nc()


@functools.lru_cache(maxsize=None)
def host_consts():
    inv_freq = (
        1.0 / (ROPE_BASE ** (np.arange(0, HD, 2, dtype=np.float32) / HD))
    ).astype(np.float32)
    freqs = np.outer(np.arange(S, dtype=np.float32), inv_freq)  # [S, 32]
    cosT = np.cos(freqs).T.astype(np.float32)  # [32, S]
    sinT = np.sin(freqs).T.astype(np.float32)
    cos2 = np.ascontiguousarray(np.tile(cosT, (4, 1))).astype(BF)  # [128, S]
    sin2 = np.ascontiguousarray(
        np.concatenate([sinT, -sinT, sinT, -sinT], axis=0)
    ).astype(BF)
    bd = np.zeros((P, 2), np.float32)
    bd[0:HD, 0] = 1.0
    bd[HD:P, 1] = 1.0
    ones64 = np.ones((HD, 1), np.float32)
    onesrow = np.ones((1, HD), np.float32)
    # swap matrix: out[p'] = in[swap(p')], swap flips +-32 within each 64-block
    swapm = np.zeros((P, P), np.float32)
    for pcol in range(P):
        base, off = pcol - pcol % HD, pcol % HD
        swapm[base + (off + 32) % HD, pcol] = 1.0
    dmask = (np.arange(P)[None, :] >= np.arange(P)[:, None]).astype(np.float32)
    dmask2 = np.concatenate([np.zeros((P, P), np.float32), dmask], axis=1)
    return dict(
        cos2=cos2, sin2=sin2, bd=bd.astype(BF), ones64=ones64.astype(BF),
        onesrow=onesrow.astype(BF), swapm=swapm.astype(BF),
        dmask=dmask.astype(BF), dmask2=np.ascontiguousarray(dmask2).astype(BF),
    )


def make_in_maps(x, w_q, w_k, w_v, w_proj, q_gain, n_cores=N_CORES, group_size=G):
    consts = host_consts()
    xT_b = [
        np.ascontiguousarray(np.asarray(x[b]).T).astype(BF) for b in range(B)
    ]
    in_maps = []
    for core in range(n_cores):
        b, g = core // group_size, core % group_size
        wkv = np.concatenate(
            [w_k[g * HD : (g + 1) * HD, :], w_v[g * HD : (g + 1) * HD, :]], axis=0
        )
        # gains/8 baked into the rms-broadcast lhsT rows (one per head)
        bd2q = np.zeros((4, P), np.float32)
        for i in range(4):
            hcols = slice((i % 2) * HD, (i % 2 + 1) * HD)
            bd2q[i, hcols] = q_gain[4 * g + i] * 0.125
        in_maps.append(
            dict(
                xT=xT_b[b],
                wq=np.ascontiguousarray(w_q[g * QROWS : (g + 1) * QROWS, :]),
                wkv=np.ascontiguousarray(wkv),
                wp=np.ascontiguousarray(w_proj[g * QROWS : (g + 1) * QROWS, :]),
                bd2q=bd2q.astype(BF),
                **consts,
            )
        )
    return in_maps


def assemble(results, n_cores=N_CORES, group_size=G):
    out = np.empty((B, S, D), np.float32)
    for core in range(n_cores):
        b, g = core // group_size, core % group_size
        out[b, :, g * QROWS : (g + 1) * QROWS] = results[core]["out"].astype(
            np.float32
        )
    return out


def kernel(**inputs):
    x = np.asarray(inputs["x"], np.float32)
    w_q = np.asarray(inputs["w_q"], np.float32)
    w_k = np.asarray(inputs["w_k"], np.float32)
    w_v = np.asarray(inputs["w_v"], np.float32)
    w_proj = np.asarray(inputs["w_proj"], np.float32)
    q_gain = np.asarray(inputs["q_gain"], np.float32)

    nc = get_nc()
    in_maps = make_in_maps(x, w_q, w_k, w_v, w_proj, q_gain)
    res = run_bass_kernel_spmd(nc, in_maps, list(range(N_CORES)))
    return assemble(res.results)
